# revision 61
# baseline (speedup 1.0000x reference)
"""EnsembleGCN Trainium2 kernel v4 — 8-core SPMD Bass implementation.

Sharding: core b owns node rows [256b, 256b+256).

Structure (one collective g2 + two half-collectives g3a/g3b):
  - The input GCNs' symmetric normalization D^-1/2 (A+I) D^-1/2 is
    folded into the host-prepped wsl matrices (input-only data), so no
    degree collective is needed for them.
  - Head: Z = X @ W (pz) then te^T = Z^T @ wsl (pte) with the two
    branches' accumulation chains interleaved across two PSUM banks.
    All big operand loads are single plain-2D DMAs from host
    pre-arranged [128, x] layouts (a 3D-AP DMA costs ~3ns per
    descriptor = per contiguous run in DGE descriptor generation).
  - Distance phase: in0 is ONE [128, 2048] tile (64 fd dims replicated
    2x on partitions) built by a PE broadcast matmul from the gathered
    fdT. Per slab column c: |in0 - slab[:,c]| via DVE (tensor_scalar
    subtract + int16 bitwise-AND abs, 4x mode) or ACT (fused
    Abs-with-bias, fp8 out); ACT's fp8 tiles are consumed in
    consecutive-c pairs by DoubleRow fp8 matmuls. DVE, ACT and PE are
    all ~90% busy here (the joint roofline).
  - fd values are pre-scaled by S=64 so |diff| lands in fp8's normal
    range; the uniform 1/S scale of Ahw cancels in the normalization.
  - aw rows are bf16: the M1 diag mask multiply runs BEFORE the degree
    rowsum (diag becomes ~1/S, no large-number DIAG_CORR cancellation)
    and doubles as the f32->bf16 convert of the recip output.
  - f=1 rows flow per-512-chunk through recip -> M1 -> ACT accum_out
    rowsum so the g3b degree collective triggers right after the last
    chunk; g3a (f=0 degrees) fires mid-distance, fully hidden.
  - Finale: aw_cols via PE transposes (XBAR DMA transpose costs ~1.2us
    sequencer time each - measured net loss), psum->sbuf copies
    alternate DVE/ACT, transpose psum rotates over pgen AND the dead
    psz banks (depth 4). Even j-tiles first: the h=0 half of the final
    po matmul gates only on the early-hidden g3a collective.
"""

import numpy as np

N = 2048
BLK = 256            # rows per core
NCORE = 8
D = 256              # input feature dim
ET = 32              # embed dim t/f
EC = 128             # embed dim c
C = 5                # classes
FD = 64              # dist feature dim (te|fe)
NJT = N // 128       # 16 j-tiles
NCHUNK = 4           # 512-wide j-chunks
SSC = 64.0           # fd pre-scale for fp8 range
EPS = 1e-5
import ml_dtypes as _mld
_SEPS = float(np.float32(_mld.bfloat16(SSC * EPS)))
# raw diag value after recip is 1/_SEPS; true diag is 1/SSC
DIAG_CORR = 1.0 / _SEPS - 1.0 / SSC

# per-fill engine assignment: singles (1 column) + fp8 pairs (2 columns)
# (gpsimd cannot run tensor_scalar on TRN2 hw - codegen engine check;
#  dual-op tensor_scalar subtract+abs_max is invalid DVE ISA)
DB_N, D8_N, A8_N, G8_N = 36, 0, 24, 4     # 40 singles + 12 pairs = 64 c
FUSE = "two"    # "two": sub + int16-AND (proven); "stt": fused via STT
G8FP8 = False   # unusable: gpsimd rejected by codegen
PODR = False    # final po matmul in fp8 DoubleRow
NOB = False     # avoid stride-0 broadcast APs (fallback if codegen rejects)
SAFE = False    # all-bf16 distance tiles (no fp8/DoubleRow) for debugging
DB_N8, D8_N8, A8_N8, G8_N8 = 26, 11, 16, 11   # counts when G8FP8

_CACHE = {}


def _spread(counts):
    items = []
    for name, n in counts.items():
        if n > 0:
            items += [((i + 0.5) / n, name) for i in range(n)]
    return [name for _, name in sorted(items)]


def _fill_pattern():
    """Units: ('s', eng) or ('p', engA, engB). The first few units are
    DVE-only so the PE isn't head-of-line blocked while ACT/GPSIMD build
    their first (slower) tiles."""
    if SAFE:
        singles = [('s', e) for e in _spread({'db': DB_N + D8_N - 2,
                                              'ab': A8_N})]
        seq = [('s', 'db'), ('s', 'db')]
        seq += singles
        return seq
    if G8FP8:
        fp8list = _spread({'d8': D8_N8 - 2, 'a8': A8_N8, 'g8': G8_N8})
        pairs = [('p', fp8list[2 * i], fp8list[2 * i + 1])
                 for i in range((D8_N8 - 2 + A8_N8 + G8_N8) // 2)]
        singles = [('s', 'db') for _ in range(DB_N8 - 2)]
    else:
        d8rest = max(D8_N - 2, 0)
        fp8list = _spread({'d8': d8rest, 'a8': A8_N})
        pairs = [('p', fp8list[2 * i], fp8list[2 * i + 1])
                 for i in range((d8rest + A8_N) // 2)]
        singles = [('s', e) for e in _spread({'db': DB_N - 2, 'gb': G8_N})]
    merged = _spread({'S': len(singles), 'P': len(pairs)})
    si = pi = 0
    seq = [('s', 'db'), ('s', 'db')]
    if D8_N >= 2 and not G8FP8:
        seq.insert(1, ('p', 'd8', 'd8'))
    for u in merged:
        if u == 'S':
            seq.append(singles[si]); si += 1
        else:
            seq.append(pairs[pi]); pi += 1
    return seq


def _build_program():
    import concourse.bass as bass
    import concourse.tile as tile
    from concourse import bacc, mybir
    from contextlib import ExitStack
    f32 = mybir.dt.float32
    bf16 = mybir.dt.bfloat16
    fp8 = mybir.dt.float8e4
    Alu = mybir.AluOpType
    Act = mybir.ActivationFunctionType
    DR = mybir.MatmulPerfMode.DoubleRow

    nc = bacc.Bacc(None, target_bir_lowering=False, debug=False,
                   num_devices=NCORE)

    dp = nc.declare_dram_parameter
    # host pre-arranges wsl as [128, NJT*BLK] (partition-major) so the
    # load is a plain 2D DMA: the 3D-AP form generated 2048 descriptors
    # (~6.4us of DGE descriptor generation on the sync queue)
    wsl_t = dp("wsl_t", [128, NJT * BLK], bf16, isOutput=False)
    wsl_f = dp("wsl_f", [128, NJT * BLK], bf16, isOutput=False)
    # xtw = [X^T | W], host pre-arranged to [128, 2*(N+ET)] for a plain
    # 2D one-descriptor-per-partition DMA
    xtw_t = dp("xtw_t", [128, 2 * (N + ET)], bf16, isOutput=False)
    xtw_f = dp("xtw_f", [128, 2 * (N + ET)], bf16, isOutput=False)
    wc = dp("wc", [FD + C, EC], bf16, isOutput=False)
    wo = dp("wo", [EC, C], bf16, isOutput=False)
    ohT = dp("ohT", [C, N], bf16, isOutput=False)
    # packed constant blobs (one DMA each):
    #   cb128 bf16: i128b[0:128] | onesb[128] | sel64[129:257] | ebig[257:511]
    #   cf128 f32:  i128f[0:128] | bc[128] | bo[129] | b_t[130] | b_f[131]
    #   crow bf16:  ones1b[0:128] | epsrow[128:128+N]
    #   crowf f32:  onesrow[0:128]
    cb128 = dp("cb128", [128, 511], bf16, isOutput=False)
    cf128 = dp("cf128", [128, 132], f32, isOutput=False)
    crow = dp("crow", [1, 128 + N], bf16, isOutput=False)
    crowf = dp("crowf", [1, 128], f32, isOutput=False)
    ebig8 = dp("ebig8", [128, 384], fp8, isOutput=False)
    m1 = dp("m1", [128, 2 * N], bf16, isOutput=False)   # per-core diag mask
    y = dp("y", [C, BLK], f32, isOutput=True)

    g2_in = nc.dram_tensor("g2_in", [FD, BLK], bf16)
    g2_out = nc.dram_tensor("g2_out", [FD * NCORE, BLK], bf16,
                            addr_space="Shared")
    # degree AllGather split into per-f-half collectives: the f=0 half
    # fires mid-distance (fully hidden), only the f=1 half sits on the
    # critical path
    g3h_in = []
    g3h_out = []
    for h in range(2):
        g3h_in.append(nc.dram_tensor(f"g3{h}_in", [1, 128], f32))
        g3h_out.append(nc.dram_tensor(f"g3{h}_out", [NCORE, 128], f32,
                                      addr_space="Shared"))

    RG = [list(range(NCORE))]

    with tile.TileContext(nc) as tc, ExitStack() as ex:
        cst = ex.enter_context(tc.tile_pool(name="cst", bufs=1))
        big = ex.enter_context(tc.tile_pool(name="big", bufs=1))
        wk = ex.enter_context(tc.tile_pool(name="wk", bufs=2))
        tpd = ex.enter_context(tc.tile_pool(name="tpd", bufs=5))
        tpg = ex.enter_context(tc.tile_pool(name="tpg", bufs=3))
        tp8 = ex.enter_context(tc.tile_pool(name="tp8", bufs=6))
        ps = ex.enter_context(tc.tile_pool(name="ps", bufs=2, space="PSUM"))
        psz = ex.enter_context(tc.tile_pool(name="psz", bufs=1, space="PSUM"))
        psd = ex.enter_context(tc.tile_pool(name="psd", bufs=1, space="PSUM"))

        # ---- small constant blob first (onesb unblocks colsum), then wsl
        cb = cst.tile([128, 511], bf16)
        nc.sync.dma_start(cb[:], cb128[:])
        # Z = X @ W; ONE 3D-AP DMA per branch, one branch per DGE queue
        # so both land ~in parallel with the wsl loads
        xtw_sb = {}
        for m, xtw in enumerate([xtw_t, xtw_f]):
            t = big.tile([128, 2 * (N + ET)], bf16, tag=f"xtw{m}")
            nc.scalar.dma_start(t[:], xtw[:])
            xtw_sb[m] = t

        # one [128, NJT*BLK] tile + ONE DMA per branch: 32 separate DMAs
        # cost ~600ns of DGE descriptor generation EACH on the sync queue
        # (~20us of serialization) and give the pte chain 16 staggered
        # semaphore deps
        wsl_sb = {}
        for m, wsl in enumerate([wsl_t, wsl_f]):
            t = big.tile([128, NJT * BLK], bf16, tag=f"wsl{m}")
            nc.sync.dma_start(t[:], wsl[:])
            wsl_sb[m] = [t[:, kt * BLK:(kt + 1) * BLK] for kt in range(NJT)]

        i128b_sb = cb[:, 0:128]
        onesb_sb = cb[:, 128:129]
        sel64_sb = cb[0:FD, 129:257]
        ebig_sb = cb[:, 257:511]
        crf = cst.tile([1, 128], f32)
        nc.sync.dma_start(crf[:], crowf[:])
        onesrow_sb = crf[:, 0:128]

        # ---- remaining packed constants (ACT queue, behind xtw) ----
        cf = cst.tile([128, 132], f32)
        nc.scalar.dma_start(cf[:], cf128[:])
        i128f_sb = cf[:, 0:128]
        bias_c = cf[:, 128:129]
        bias_o = cf[0:C, 129:130]
        bias_tf = [cf[0:ET, 130:131], cf[0:ET, 131:132]]
        cr = cst.tile([1, 128 + N], bf16)
        nc.scalar.dma_start(cr[:], crow[:])
        ones1b_sb = cr[:, 0:128]
        epsrow_sb = cr[:, 128:128 + N]
        ebig8_sb = cst.tile([128, 384], fp8)
        nc.scalar.dma_start(ebig8_sb[:], ebig8[:])

        # PE pstate warm-up: dummy matmuls on a memset tile (no DMA wait,
        # so the warm-up finishes before the first real operands land)
        wsrc = wk.tile([128, 128], bf16, tag="wsrc")
        nc.vector.memset(wsrc[:], 1.0)
        for wi in range(10):
            pwu = ps.tile([128, 128], f32, tag="pgen")
            nc.tensor.matmul(pwu[:], wsrc[:], wsrc[:],
                             start=True, stop=True)

        # warm the Lrelu ACT table while DMAs land (hides the ~1.3us
        # table load that would otherwise precede the te activation)
        lwarm = wk.tile([1, 1], f32, tag="lwarm")
        nc.scalar.activation(lwarm[:], crf[:, 0:1], Act.Lrelu, alpha=0.01)

        pz_all = {}
        for m in range(2):
            xts = [xtw_sb[m][:, 0:N], xtw_sb[m][:, N + ET:2 * N + ET]]
            wms = [xtw_sb[m][:, N:N + ET],
                   xtw_sb[m][:, 2 * N + ET:2 * (N + ET)]]
            pz = psz.tile([128, NJT * ET], f32, tag=f"pz{m}")
            for jt in range(NJT):
                sl = slice(jt * ET, (jt + 1) * ET)
                for k in range(2):
                    nc.tensor.matmul(pz[:, sl],
                                     xts[k][:, jt * 128:(jt + 1) * 128],
                                     wms[k], start=(k == 0), stop=(k == 1))
            pz_all[m] = pz

        # te^T / fe^T [32, 256] bf16 -> g2_in
        # (adjacency normalization is folded into wsl on the host, so
        #  te = lrelu(Z^T @ wsl + b) directly); the two branches'
        # accumulation chains interleave across two PSUM banks so the
        # PE pipelines them instead of stalling on each accumulate
        zbs = {}
        for m in range(2):
            zbc = []
            for q in range(4):
                zq = big.tile([128, 4 * ET], bf16, tag=f"zb{m}_{q}")
                nc.vector.tensor_copy(zq[:],
                                      pz_all[m][:, q * 4 * ET:
                                                 (q + 1) * 4 * ET])
                zbc.append(zq)
            zbs[m] = zbc
        pte0 = ps.tile([ET, BLK], f32, tag="pgen")
        pte1 = ps.tile([ET, BLK], f32, tag="pgen")
        ptes = [pte0, pte1]
        for kt in range(NJT):
            for m in range(2):
                nc.tensor.matmul(ptes[m][:],
                                 zbs[m][kt // 4][:, (kt % 4) * ET:
                                                 (kt % 4 + 1) * ET],
                                 wsl_sb[m][kt],
                                 start=(kt == 0), stop=(kt == NJT - 1))
        te_sb = []
        for m in range(2):
            te = wk.tile([ET, BLK], bf16, tag=f"teT{m}")
            nc.scalar.activation(te[:], ptes[m][:], Act.Lrelu,
                                 bias=bias_tf[m], alpha=0.01)
            te_sb.append(te)
            nc.sync.dma_start(g2_in[m * ET:(m + 1) * ET, :], te[:])

        # slab from OWN te/fe (read back from g2_in DRAM, straight
        # slices). slabW[p = r*64 + m*32 + e, q] = g2_in[m*32 + e, q]
        slabW = cst.tile([128, BLK], bf16)
        for r in range(2):
            nc.sync.dma_start(slabW[r * FD:(r + 1) * FD, :], g2_in[:])
        # slab32b[p, c] = bf16(S * fd_own[k, 2c+r])  — bf16-rounded so the
        # diagonal |in0 - slab| is exactly 0 (in0 is bf16(S*fd) too).
        slab32 = cst.tile([128, 128], bf16)
        for r in range(2):
            half = slice(r * 64, (r + 1) * 64)
            src = slabW[half, :].rearrange("k (c r2) -> r2 k c", r2=2)[r]
            nc.vector.tensor_scalar(slab32[half, :], src, SSC, None, Alu.mult)
        slabf = cst.tile([128, 128], f32)
        nc.vector.tensor_copy(slabf[:], slab32[:])
        negslab = cst.tile([128, 128], f32)
        nc.vector.tensor_scalar(negslab[:], slab32[:], -1.0, None, Alu.mult)

        # eps seeds for f=0 (constants only — run during the g2 window)
        psd_f0 = []
        for ch in range(NCHUNK):
            pcht = psd.tile([128, 512], f32, tag=f"pd{ch}")
            psd_f0.append(pcht)
            sl = slice(ch * 512, (ch + 1) * 512)
            nc.tensor.matmul(pcht[:], ones1b_sb, epsrow_sb[0:1, sl],
                             start=True, stop=False)

        nc.gpsimd.collective_compute(
            "AllGather", Alu.bypass, replica_groups=RG,
            ins=[g2_in.ap().opt()], outs=[g2_out.ap().opt()])

        # fdT in 4 separate chunk tiles so in0 chunk ch only waits its DMA
        fdT_view = g2_out.ap().rearrange("(c k) q -> k c q", c=NCORE, k=FD)
        fdT_ch = []
        for ch in range(NCHUNK):
            t = big.tile([128, 512], bf16, tag=f"fdT{ch}")
            nc.sync.dma_start(t[:FD, :], fdT_view[:, 2 * ch:2 * ch + 2, :])
            nc.sync.dma_start(t[FD:FD + C, :], ohT[:, ch * 512:(ch + 1) * 512])
            fdT_ch.append(t)
        m1t = cst.tile([128, 2 * N], bf16)
        nc.scalar.dma_start(m1t[:], m1[:])
        m1_sb = [m1t[:, 0:N], m1t[:, N:2 * N]]

        # in0[p = r*64+k, j] = bf16(S * fd[j, k]) via PE broadcast;
        # scale by S during the PSUM->SBUF copy
        in0 = big.tile([128, N], bf16, tag="in0")
        for ch in range(NCHUNK):
            sl = slice(ch * 512, (ch + 1) * 512)
            pin = ps.tile([128, 512], f32, tag="pgen")
            nc.tensor.matmul(pin[:], sel64_sb, fdT_ch[ch][:FD, :],
                             start=True, stop=True)
            # alternate the scale-copy across ACT/DVE so the four chunk
            # copies run pairwise in parallel (distance start gates on in0)
            if ch % 2 == 0:
                nc.scalar.activation(in0[:, sl], pin[:], Act.Copy, bias=0.0,
                                     scale=SSC)
            else:
                nc.vector.tensor_scalar(in0[:, sl], pin[:], SSC, None,
                                        Alu.mult)

        # ---- distance row-blocks ----
        i16 = mybir.dt.int16
        zer = cst.tile([128, N], bf16)
        if FUSE == "stt" or D8_N > 0:
            nc.vector.memset(zer[:], 0.0)

        def emit_dve_abs(dst, cg):
            if FUSE == "stt":
                # |in0 - s| = abs_max(in0 - s, 0) via scalar-tensor-tensor
                nc.vector.scalar_tensor_tensor(
                    dst, in0[:], slabf[:, cg:cg + 1], zer[:],
                    Alu.subtract, Alu.abs_max)
            else:
                nc.vector.tensor_scalar(dst, in0[:], slabf[:, cg:cg + 1],
                                        None, Alu.subtract)
                nc.vector.tensor_scalar(dst.bitcast(i16), dst.bitcast(i16),
                                        0x7FFF, None, Alu.bitwise_and)

        pattern = _fill_pattern()
        dwrow = big.tile([1, BLK], f32, tag="dwrow")
        # aw_rows in bf16: the diagonal is M1-corrected to ~1/S BEFORE the
        # rowsum (no large-number DIAG_CORR cancellation), so deg survives
        # bf16 and the 16-bit XBAR DMA transpose becomes legal
        aw_rows = []
        for f in range(2):
            awr_t = big.tile([128, N], bf16, tag=f"awr{f}")
            aw_rows.append(awr_t)
        aw_cols = []
        if PODR:
            for ktp in range(NJT // 2):
                awc_t = big.tile([128, 2 * BLK], fp8, tag=f"awc{ktp}")
                aw_cols.append(awc_t)
        else:
            for kt in range(NJT):
                awc_t = big.tile([128, BLK], bf16, tag=f"awc{kt}")
                aw_cols.append(awc_t)
        dwcol = []

        def emit_deg(f, dwr_=None):
            # deg rowsum; the diag is already M1-corrected to ~1/S, so the
            # sum is exactly (deg+1)/S — no correction constant needed.
            # For f=1 the caller passes per-chunk-accumulated partials.
            # f=0 (mid-distance) rides on ACT accum_out — DVE is the
            # binding engine there (97.9% busy) while ACT has slack; the
            # copy target is the dead f0 aw32 tile (no fill-pool theft).
            if dwr_ is None:
                dwr_ = big.tile([128, 1], f32, tag=f"dwcr{f}")
                nc.scalar.activation(aw32_f[f][:], aw_rows[f][:], Act.Copy,
                                     accum_out=dwr_[:])
            pr = ps.tile([1, 128], f32, tag="pgen")
            nc.tensor.matmul(pr[:], dwr_[:], i128f_sb, start=True, stop=True)
            nc.vector.tensor_copy(dwrow[:, f * 128:(f + 1) * 128], pr[:])
            nc.sync.dma_start(g3h_in[f][:], dwrow[:, f * 128:(f + 1) * 128])
            nc.gpsimd.collective_compute(
                "AllGather", Alu.bypass, replica_groups=RG,
                ins=[g3h_in[f].ap().opt()], outs=[g3h_out[f].ap().opt()])

        aw32_f = {}

        def emit_m1(f):
            # M1 diag fix; also performs the f32 -> bf16 conversion of
            # the recip output (recip itself must write fp32).
            # f=0 runs mid-distance where DVE is the binding engine
            # (~98% busy) — run it on the otherwise idle GPSIMD there.
            eng = nc.gpsimd if f == 0 else nc.vector
            eng.tensor_tensor(aw_rows[f][:], aw32_f[f][:],
                              m1_sb[f], Alu.mult)

        def emit_unit(unit, c, f, pch, is_last):
            cg = f * 64 + c
            if unit[0] == 's':
                dve = unit[1] == 'db'
                pool = tpd if dve else tpg
                t = pool.tile([128, N], bf16, tag="tb")
                if dve:
                    emit_dve_abs(t[:], cg)
                elif unit[1] == 'gb':
                    # gpsimd subtract (slab col broadcast along free via
                    # stride-0 AP) + DVE int16-AND abs: offloads the sub
                    # from the binding DVE engine to the idle GPSIMD
                    sv = slabf[:]
                    slabb = bass.AP(tensor=sv.tensor,
                                    offset=sv.offset + cg,
                                    ap=[[128, 128], [0, N]])
                    nc.gpsimd.tensor_tensor(t[:], in0[:], slabb,
                                            Alu.subtract)
                    nc.vector.tensor_scalar(t[:].bitcast(i16),
                                            t[:].bitcast(i16),
                                            0x7FFF, None, Alu.bitwise_and)
                else:
                    nc.scalar.activation(t[:], in0[:], Act.Abs,
                                         bias=negslab[:, cg:cg + 1])
                s = 126 - 2 * c
                for ch in range(NCHUNK):
                    sl = slice(ch * 512, (ch + 1) * 512)
                    nc.tensor.matmul(pch[ch][:], ebig_sb[:, s:s + 128],
                                     t[:, sl], start=False, stop=is_last)
                return c + 1
            # fp8 pair at columns c, c+1
            t8 = tp8.tile([128, 2 * N], fp8, tag="t8")
            for h, eng in enumerate(unit[1:3]):
                half = t8[:, h * N:(h + 1) * N]
                cgh = cg + h
                if eng == 'd8':
                    nc.vector.scalar_tensor_tensor(
                        half, in0[:], slabf[:, cgh:cgh + 1], zer[:],
                        Alu.subtract, Alu.abs_max)
                elif eng == 'g8':
                    nc.gpsimd.tensor_scalar(half, in0[:],
                                            slabf[:, cgh:cgh + 1], 0.0,
                                            Alu.subtract, Alu.abs_max)
                else:
                    nc.scalar.activation(half, in0[:], Act.Abs,
                                         bias=negslab[:, cgh:cgh + 1])
            s8 = 126 - 2 * c
            w8 = ebig8_sb[:, s8:s8 + 256].rearrange("p (two m) -> p two m",
                                                    two=2)
            t8v = t8[:].rearrange("p (two n) -> p two n", two=2)
            for ch in range(NCHUNK):
                nc.tensor.matmul(pch[ch][:], w8,
                                 t8v[:, :, ch * 512:(ch + 1) * 512],
                                 start=False, stop=is_last,
                                 perf_mode=DR)
            return c + 2

        for f in range(2):
            if f == 0:
                pch = psd_f0
            else:
                pch = []
                for ch in range(NCHUNK):
                    # chunks 0-1 reuse the psz banks (dead since the
                    # head), so their eps seeds need not wait f0 recips
                    if ch < 2:
                        pcht = psz.tile([128, 512], f32, tag=f"pz{ch}")
                    else:
                        pcht = psd.tile([128, 512], f32, tag=f"pd{ch - 2}")
                    pch.append(pcht)
                # eps seed: psum = S*EPS everywhere (start=True)
                for ch in range(NCHUNK):
                    sl = slice(ch * 512, (ch + 1) * 512)
                    nc.tensor.matmul(pch[ch][:], ones1b_sb,
                                     epsrow_sb[0:1, sl],
                                     start=True, stop=False)
            c = 0
            for ui, unit in enumerate(pattern):
                c = emit_unit(unit, c, f, pch, ui == len(pattern) - 1)
                if f == 1 and ui == len(pattern) // 3:
                    emit_m1(0)
                    emit_deg(0)
            assert c == 64
            # 1/(S*(dist+eps)) = Ahw/S rows (per-chunk: next fill's eps
            # seed of chunk ch only waits recip of chunk ch)
            aw32 = big.tile([128, N], f32, tag="aw32")
            aw32_f[f] = aw32
            if f == 0:
                for ch in range(NCHUNK):
                    sl = slice(ch * 512, (ch + 1) * 512)
                    nc.vector.reciprocal_approx_fast(out=aw32[:, sl],
                                                     in_=pch[ch][:])
            else:
                # per-chunk recip -> M1 -> ACT rowsum pipeline so the g3b
                # trigger fires right after the last chunk instead of
                # after two serial full-width ops
                dwp = big.tile([128, NCHUNK], f32, tag="dwp")
                for ch in range(NCHUNK):
                    sl = slice(ch * 512, (ch + 1) * 512)
                    nc.vector.reciprocal_approx_fast(out=aw32[:, sl],
                                                     in_=pch[ch][:])
                    nc.vector.tensor_tensor(aw_rows[1][:, sl],
                                            aw32[:, sl],
                                            m1_sb[1][:, sl], Alu.mult)
                    scr = tpd.tile([128, 512], bf16, tag="tb")
                    nc.scalar.activation(scr[:], aw_rows[1][:, sl],
                                         Act.Copy,
                                         accum_out=dwp[:, ch:ch + 1])
                dwr1 = big.tile([128, 1], f32, tag="dwcr1")
                nc.vector.tensor_reduce(dwr1[:], dwp[:],
                                        mybir.AxisListType.X, Alu.add)
                emit_deg(1, dwr1)

        # dwr128 broadcast from the local deg row (overlaps g3)
        rcr = wk.tile([1, BLK], f32, tag="rcr")
        nc.vector.reciprocal_approx_fast(out=rcr[:], in_=dwrow[:])
        dwr = wk.tile([1, BLK], f32, tag="dwr")
        nc.scalar.activation(dwr[:], rcr[:], Act.Sqrt)
        pb128 = ps.tile([128, BLK], f32, tag="pgen")
        nc.tensor.matmul(pb128[:], onesrow_sb, dwr[:],
                         start=True, stop=True)
        dwr128 = big.tile([128, BLK], f32, tag="dwr128")
        nc.vector.tensor_copy(dwr128[:], pb128[:])

        # transposes -> aw_cols plus F' = feats @ Wc (PE idles during
        # g3b). Even j-tiles first: they feed the h=0 half of po, which
        # only waits on the early hidden g3a collective. The psum->sbuf
        # copies alternate DVE/ACT so the copy tail runs on two engines.
        wc_sb = cst.tile([FD + C, EC], bf16)
        nc.sync.dma_start(wc_sb[:], wc[:])
        fp_all = big.tile([128, NJT * EC], bf16, tag="fpall")

        def emit_cols(kt, alt):
            # PE transposes (XBAR DMA transpose costs ~1.2us of
            # sequencer-side descriptor generation each — measured net
            # loss); psum->sbuf copies alternate DVE/ACT
            for f in range(2):
                # rotate transpose psum across pgen AND the dead psz
                # banks: depth-4 pipeline instead of depth-2
                if f == 0:
                    pt = ps.tile([128, 128], bf16, tag="pgen")
                else:
                    pt = psz.tile([128, 128], bf16, tag=f"pz{kt % 2}")
                nc.tensor.transpose(pt[:],
                                    aw_rows[f][:, kt * 128:(kt + 1) * 128],
                                    i128b_sb)
                dst = aw_cols[kt][:, f * 128:(f + 1) * 128]
                if (alt + f) % 2 == 0:
                    nc.vector.tensor_copy(dst, pt[:])
                else:
                    nc.scalar.copy(dst, pt[:])
            p = ps.tile([128, EC], f32, tag="pgen")
            nc.tensor.matmul(
                p[:],
                fdT_ch[kt // 4][:FD + C, (kt % 4) * 128:(kt % 4 + 1) * 128],
                wc_sb[:], start=True, stop=True)
            if alt % 2 == 0:
                nc.vector.tensor_copy(fp_all[:, kt * EC:(kt + 1) * EC], p[:])
            else:
                nc.scalar.copy(fp_all[:, kt * EC:(kt + 1) * EC], p[:])

        # deg/disw per f-half: the h=0 half only depends on the early
        # (mid-distance) g3a collective, so its 8 po matmuls run before
        # g3b even lands; only the h=1 half waits for g3b. Emission
        # order (even cols -> h0 chain -> odd cols -> h1 chain) keeps
        # the h0 chain ahead of the odd copies in every engine queue.
        # po lives in the psd pool (pd0's bank is free after the last
        # recip) so the pgen rotation can't recycle it mid-accumulation.
        po = psd.tile([EC, BLK], f32, tag="pd0")
        fp_v = fp_all[:].rearrange("p (c two e) -> two p c e", two=2,
                                   c=NCORE, e=EC)

        def emit_half(h):
            dgh = big.tile([128, NCORE], f32, tag=f"dgw{h}")
            nc.sync.dma_start(dgh[:],
                              g3h_out[h].ap().rearrange("c p -> p c"))
            rch = big.tile([128, NCORE], f32, tag=f"rcw{h}")
            nc.vector.reciprocal_approx_fast(out=rch[:], in_=dgh[:])
            dish = big.tile([128, NCORE], bf16, tag=f"disw{h}")
            nc.scalar.activation(dish[:], rch[:], Act.Sqrt)
            dwv = dish[:]
            rw = big.tile([128, NCORE * EC], bf16, tag=f"rw{h}")
            dwrep = bass.AP(tensor=dwv.tensor, offset=dwv.offset,
                            ap=[[NCORE, 128], [1, NCORE], [0, EC]])
            nc.vector.tensor_tensor(
                rw[:].rearrange("p (a c) -> p a c", a=NCORE),
                fp_v[h], dwrep, Alu.mult)
            for cc in range(NCORE):
                kt = 2 * cc + h
                nc.tensor.matmul(po[:], rw[:, cc * EC:(cc + 1) * EC],
                                 aw_cols[kt][:],
                                 start=(h == 0 and cc == 0),
                                 stop=(h == 1 and cc == NCORE - 1))

        for i, kt in enumerate(range(0, NJT, 2)):
            emit_cols(kt, i)
        emit_half(0)
        for i, kt in enumerate(range(1, NJT, 2)):
            emit_cols(kt, i)
        emit_half(1)
        # re-warm the Lrelu table under the po matmuls (the Sqrt set
        # load above evicted it; this keeps embT's Lrelu load off the
        # critical path)
        lw2 = wk.tile([1, 1], f32, tag="lwarm2")
        nc.scalar.activation(lw2[:], crf[:, 0:1], Act.Lrelu, alpha=0.01)
        tmp3 = wk.tile([EC, BLK], f32, tag="tmp3")
        nc.vector.tensor_tensor(tmp3[:], po[:], dwr128[:], Alu.mult)
        embT = wk.tile([EC, BLK], bf16, tag="embT")
        nc.scalar.activation(embT[:], tmp3[:], Act.Lrelu, bias=bias_c,
                             alpha=0.01)

        wo_sb = cst.tile([EC, C], bf16)
        nc.sync.dma_start(wo_sb[:], wo[:])
        ph = ps.tile([C, BLK], f32, tag="pgen")
        nc.tensor.matmul(ph[:], wo_sb[:], embT[:], start=True, stop=True)
        yout = wk.tile([C, BLK], f32, tag="yout")
        nc.vector.tensor_scalar(yout[:], ph[:], bias_o, None, Alu.add)
        nc.sync.dma_start(y[:], yout[:])

    nc.finalize()
    return nc


def _host_prep(inputs):
    import ml_dtypes
    bf = ml_dtypes.bfloat16
    f8 = ml_dtypes.float8_e4m3

    ei = np.asarray(inputs["edge_index"])
    wt = np.asarray(inputs["time_edge_weight"], np.float32)
    wf = np.asarray(inputs["freq_edge_weight"], np.float32)
    xt = np.asarray(inputs["time_features"], np.float32)
    xf = np.asarray(inputs["freq_features"], np.float32)
    labels = np.asarray(inputs["labels"])
    num_classes = int(inputs["num_classes"])
    query_size = int(inputs["query_size"])
    n = xt.shape[0]
    assert n == N and num_classes == C

    offdiag = ~np.eye(n, dtype=bool)
    r_can = np.repeat(np.arange(n, dtype=ei.dtype), n - 1)
    cgrid = np.broadcast_to(np.arange(n, dtype=ei.dtype), (n, n))
    c_can = cgrid[offdiag]
    canonical = np.array_equal(ei[0], r_can) and np.array_equal(ei[1], c_can)

    def build_wmat(w):
        # Wmat[src, dst] = w  (Wmat = A^T), plus identity
        Wm = np.zeros((n, n), np.float32)
        if canonical:
            Wm[offdiag] = w
        else:
            A = np.zeros((n, n), np.float32)
            np.add.at(A, (ei[1], ei[0]), np.asarray(w, np.float64))
            np.fill_diagonal(A, 0.0)
            Wm = np.ascontiguousarray(A.T.astype(np.float32))
        Wm[np.arange(n), np.arange(n)] = 1.0  # + I
        # fold the GCN symmetric normalization D^-1/2 (A+I) D^-1/2 in here
        # (depends only on the input edge weights; Wm is symmetric-scaled
        #  so the transpose orientation is unaffected)
        deg = Wm.sum(axis=0, dtype=np.float64)
        dis = 1.0 / np.sqrt(deg)
        return (dis[:, None] * Wm * dis[None, :]).astype(np.float32)

    Wm_t = build_wmat(wt)
    Wm_f = build_wmat(wf)

    cols = np.zeros((num_classes,), np.float32)
    cols[labels] = 1.0
    rowmask = (np.arange(n) < (n - query_size)).astype(np.float32)
    onehotT = np.ascontiguousarray(rowmask[None, :] * cols[:, None])

    # selector: in0[p = r*64+k] = fdT[k]
    s64 = np.zeros((FD, 128), np.float32)
    for r in range(2):
        s64[np.arange(FD), r * FD + np.arange(FD)] = 1.0

    up = (np.arange(128) < 64).astype(np.float32)
    dn = 1.0 - up
    eb = np.zeros((128, 254), np.float32)
    eb[:, 126] = up
    eb[:, 127] = dn
    eb8 = np.zeros((128, 384), np.float32)
    eb8[:, 126] = up
    eb8[:, 127] = dn
    eb8[:, 256] = up
    eb8[:, 257] = dn

    # diag value after recip: 1/bf16(S*EPS); M1 maps it to 1/S
    seps = np.float32(bf(SSC * EPS))
    diagfix = np.float32((1.0 / SSC) / (1.0 / seps))

    def pack_bf(parts, width):
        blob = np.zeros((128, width), np.float32)
        col = 0
        for p in parts:
            r, w = p.shape
            blob[:r, col:col + w] = p
            col += w
        assert col == width
        return blob

    # cb128: i128b | onesb | sel64 | ebig
    cbblob = pack_bf([np.eye(128, dtype=np.float32),
                      np.ones((128, 1), np.float32), s64, eb], 511)
    # cf128: i128f | bc | bo | b_t | b_f
    cfblob = pack_bf([np.eye(128, dtype=np.float32),
                      np.asarray(inputs["bc"], np.float32).reshape(EC, 1),
                      np.asarray(inputs["bo"], np.float32).reshape(C, 1),
                      np.asarray(inputs["bt"], np.float32).reshape(ET, 1),
                      np.asarray(inputs["bf"], np.float32).reshape(ET, 1)],
                     132)
    crblob = np.concatenate([np.ones((1, 128), np.float32),
                             np.full((1, N), SSC * EPS, np.float32)], axis=1)

    def xtw(x, w):
        arr = np.concatenate(
            [np.ascontiguousarray(x.T),
             np.asarray(w, np.float32)], axis=1)
        return np.ascontiguousarray(
            arr.reshape(2, 128, N + ET).transpose(1, 0, 2)
            .reshape(128, 2 * (N + ET))).astype(bf)

    shared = {
        "xtw_t": xtw(xt, inputs["Wt"]),
        "xtw_f": xtw(xf, inputs["Wf"]),
        "wc": np.asarray(inputs["Wc"], np.float32).astype(bf),
        "wo": np.asarray(inputs["Wo"], np.float32).astype(bf),
        "ohT": onehotT.astype(bf),
        "cb128": cbblob.astype(bf),
        "cf128": cfblob,
        "crow": crblob.astype(bf),
        "crowf": np.ones((1, 128), np.float32),
        "ebig8": eb8.astype(f8),
    }

    in_maps = []
    for b in range(NCORE):
        m = dict(shared)
        mm = np.ones((128, 2 * N), np.float32)
        rows = np.arange(128)
        for f in range(2):
            mm[rows, f * N + b * BLK + f * 128 + rows] = diagfix
        m["m1"] = mm.astype(bf)
        def warr(Wm):
            blk = Wm[:, b * BLK:(b + 1) * BLK].reshape(NJT, 128, BLK)
            return np.ascontiguousarray(
                blk.transpose(1, 0, 2).reshape(128, NJT * BLK)).astype(bf)
        m["wsl_t"] = warr(Wm_t)
        m["wsl_f"] = warr(Wm_f)
        in_maps.append(m)
    return in_maps


def _get_program():
    if "nc" not in _CACHE:
        _CACHE["nc"] = _build_program()
    return _CACHE["nc"]


def run(inputs, trace=False):
    from concourse.bass_utils import run_bass_kernel_spmd
    in_maps = _host_prep(inputs)
    nc = _get_program()
    res = run_bass_kernel_spmd(nc, in_maps, core_ids=list(range(NCORE)),
                               trace=trace)
    blocks = [res.results[b]["y"] for b in range(NCORE)]
    out = np.concatenate([blk.T for blk in blocks], axis=0).astype(np.float32)
    return out, res


def kernel(**inputs):
    out, _ = run(inputs, trace=False)
    return out



# revision 62
# speedup vs baseline: 1.0410x; 1.0410x over previous
"""EnsembleGCN Trainium2 kernel v4 — 8-core SPMD Bass implementation.

Sharding: core b owns node rows [256b, 256b+256).

Structure (one collective g2 + two half-collectives g3a/g3b):
  - The input GCNs' symmetric normalization D^-1/2 (A+I) D^-1/2 is
    folded into the host-prepped wsl matrices (input-only data), so no
    degree collective is needed for them.
  - Head: Z = X @ W (pz) then te^T = Z^T @ wsl (pte) with the two
    branches' accumulation chains interleaved across two PSUM banks.
    All big operand loads are single plain-2D DMAs from host
    pre-arranged [128, x] layouts (a 3D-AP DMA costs ~3ns per
    descriptor = per contiguous run in DGE descriptor generation).
  - Distance phase: in0 is ONE [128, 2048] tile (64 fd dims replicated
    2x on partitions) built by a PE broadcast matmul from the gathered
    fdT. Per slab column c: |in0 - slab[:,c]| via DVE (tensor_scalar
    subtract + int16 bitwise-AND abs, 4x mode) or ACT (fused
    Abs-with-bias, fp8 out); ACT's fp8 tiles are consumed in
    consecutive-c pairs by DoubleRow fp8 matmuls. DVE, ACT and PE are
    all ~90% busy here (the joint roofline).
  - fd values are pre-scaled by S=64 so |diff| lands in fp8's normal
    range; the uniform 1/S scale of Ahw cancels in the normalization.
  - aw rows are bf16: the M1 diag mask multiply runs BEFORE the degree
    rowsum (diag becomes ~1/S, no large-number DIAG_CORR cancellation)
    and doubles as the f32->bf16 convert of the recip output.
  - f=1 rows flow per-512-chunk through recip -> M1 -> ACT accum_out
    rowsum so the g3b degree collective triggers right after the last
    chunk; g3a (f=0 degrees) fires mid-distance, fully hidden.
  - Finale: aw_cols via PE transposes (XBAR DMA transpose costs ~1.2us
    sequencer time each - measured net loss), psum->sbuf copies
    alternate DVE/ACT, transpose psum rotates over pgen AND the dead
    psz banks (depth 4). Even j-tiles first: the h=0 half of the final
    po matmul gates only on the early-hidden g3a collective.
"""

import numpy as np

N = 2048
BLK = 256            # rows per core
NCORE = 8
D = 256              # input feature dim
ET = 32              # embed dim t/f
EC = 128             # embed dim c
C = 5                # classes
FD = 64              # dist feature dim (te|fe)
NJT = N // 128       # 16 j-tiles
NCHUNK = 4           # 512-wide j-chunks
SSC = 64.0           # fd pre-scale for fp8 range
EPS = 1e-5
import ml_dtypes as _mld
_SEPS = float(np.float32(_mld.bfloat16(SSC * EPS)))
# raw diag value after recip is 1/_SEPS; true diag is 1/SSC
DIAG_CORR = 1.0 / _SEPS - 1.0 / SSC

# per-fill engine assignment: singles (1 column) + fp8 pairs (2 columns)
# (gpsimd cannot run tensor_scalar on TRN2 hw - codegen engine check;
#  dual-op tensor_scalar subtract+abs_max is invalid DVE ISA)
DB_N, D8_N, A8_N, G8_N = 38, 0, 26, 0     # 38 singles + 13 pairs = 64 c
FUSE = "two"    # "two": sub + int16-AND (proven); "stt": fused via STT
G8FP8 = False   # unusable: gpsimd rejected by codegen
PODR = False    # final po matmul in fp8 DoubleRow
NOB = False     # avoid stride-0 broadcast APs (fallback if codegen rejects)
SAFE = False    # all-bf16 distance tiles (no fp8/DoubleRow) for debugging
DB_N8, D8_N8, A8_N8, G8_N8 = 26, 11, 16, 11   # counts when G8FP8

_CACHE = {}


def _spread(counts):
    items = []
    for name, n in counts.items():
        if n > 0:
            items += [((i + 0.5) / n, name) for i in range(n)]
    return [name for _, name in sorted(items)]


def _fill_pattern():
    """Units: ('s', eng) or ('p', engA, engB). The first few units are
    DVE-only so the PE isn't head-of-line blocked while ACT/GPSIMD build
    their first (slower) tiles."""
    if SAFE:
        singles = [('s', e) for e in _spread({'db': DB_N + D8_N - 2,
                                              'ab': A8_N})]
        seq = [('s', 'db'), ('s', 'db')]
        seq += singles
        return seq
    if G8FP8:
        fp8list = _spread({'d8': D8_N8 - 2, 'a8': A8_N8, 'g8': G8_N8})
        pairs = [('p', fp8list[2 * i], fp8list[2 * i + 1])
                 for i in range((D8_N8 - 2 + A8_N8 + G8_N8) // 2)]
        singles = [('s', 'db') for _ in range(DB_N8 - 2)]
    else:
        d8rest = max(D8_N - 2, 0)
        fp8list = _spread({'d8': d8rest, 'a8': A8_N})
        pairs = [('p', fp8list[2 * i], fp8list[2 * i + 1])
                 for i in range((d8rest + A8_N) // 2)]
        singles = [('s', e) for e in _spread({'db': DB_N - 2, 'gb': G8_N})]
    merged = _spread({'S': len(singles), 'P': len(pairs)})
    si = pi = 0
    seq = [('s', 'db'), ('s', 'db')]
    if D8_N >= 2 and not G8FP8:
        seq.insert(1, ('p', 'd8', 'd8'))
    for u in merged:
        if u == 'S':
            seq.append(singles[si]); si += 1
        else:
            seq.append(pairs[pi]); pi += 1
    return seq


def _build_program():
    import concourse.bass as bass
    import concourse.tile as tile
    from concourse import bacc, mybir
    from contextlib import ExitStack
    f32 = mybir.dt.float32
    bf16 = mybir.dt.bfloat16
    fp8 = mybir.dt.float8e4
    Alu = mybir.AluOpType
    Act = mybir.ActivationFunctionType
    DR = mybir.MatmulPerfMode.DoubleRow

    nc = bacc.Bacc(None, target_bir_lowering=False, debug=False,
                   num_devices=NCORE)

    dp = nc.declare_dram_parameter
    # host pre-arranges wsl as [128, NJT*BLK] (partition-major) so the
    # load is a plain 2D DMA: the 3D-AP form generated 2048 descriptors
    # (~6.4us of DGE descriptor generation on the sync queue)
    wsl_t = dp("wsl_t", [128, NJT * BLK], bf16, isOutput=False)
    wsl_f = dp("wsl_f", [128, NJT * BLK], bf16, isOutput=False)
    # xtw = [X^T | W], host pre-arranged to [128, 2*(N+ET)] for a plain
    # 2D one-descriptor-per-partition DMA
    xtw_t = dp("xtw_t", [128, 2 * (N + ET)], bf16, isOutput=False)
    xtw_f = dp("xtw_f", [128, 2 * (N + ET)], bf16, isOutput=False)
    wc = dp("wc", [FD + C, EC], bf16, isOutput=False)
    wo = dp("wo", [EC, C], bf16, isOutput=False)
    ohT = dp("ohT", [C, N], bf16, isOutput=False)
    # packed constant blobs (one DMA each):
    #   cb128 bf16: i128b[0:128] | onesb[128] | sel64[129:257] | ebig[257:511]
    #   cf128 f32:  i128f[0:128] | bc[128] | bo[129] | b_t[130] | b_f[131]
    #   crow bf16:  ones1b[0:128] | epsrow[128:128+N]
    #   crowf f32:  onesrow[0:128]
    cb128 = dp("cb128", [128, 511], bf16, isOutput=False)
    cf128 = dp("cf128", [128, 132], f32, isOutput=False)
    crow = dp("crow", [1, 128 + N], bf16, isOutput=False)
    crowf = dp("crowf", [1, 128], f32, isOutput=False)
    ebig8 = dp("ebig8", [128, 384], fp8, isOutput=False)
    m1 = dp("m1", [128, 2 * N], bf16, isOutput=False)   # per-core diag mask
    y = dp("y", [C, BLK], f32, isOutput=True)

    g2_in = nc.dram_tensor("g2_in", [FD, BLK], bf16)
    g2_out = nc.dram_tensor("g2_out", [FD * NCORE, BLK], bf16,
                            addr_space="Shared")
    # degree AllGather split into per-f-half collectives: the f=0 half
    # fires mid-distance (fully hidden), only the f=1 half sits on the
    # critical path
    g3h_in = []
    g3h_out = []
    for h in range(2):
        g3h_in.append(nc.dram_tensor(f"g3{h}_in", [1, 128], f32))
        g3h_out.append(nc.dram_tensor(f"g3{h}_out", [NCORE, 128], f32,
                                      addr_space="Shared"))

    RG = [list(range(NCORE))]

    with tile.TileContext(nc) as tc, ExitStack() as ex:
        cst = ex.enter_context(tc.tile_pool(name="cst", bufs=1))
        big = ex.enter_context(tc.tile_pool(name="big", bufs=1))
        wk = ex.enter_context(tc.tile_pool(name="wk", bufs=2))
        tpd = ex.enter_context(tc.tile_pool(name="tpd", bufs=5))
        tpg = ex.enter_context(tc.tile_pool(name="tpg", bufs=3))
        tp8 = ex.enter_context(tc.tile_pool(name="tp8", bufs=6))
        ps = ex.enter_context(tc.tile_pool(name="ps", bufs=2, space="PSUM"))
        psz = ex.enter_context(tc.tile_pool(name="psz", bufs=1, space="PSUM"))
        psd = ex.enter_context(tc.tile_pool(name="psd", bufs=1, space="PSUM"))

        # ---- small constant blob first (onesb unblocks colsum), then wsl
        cb = cst.tile([128, 511], bf16)
        nc.sync.dma_start(cb[:], cb128[:])
        # Z = X @ W; ONE 3D-AP DMA per branch, one branch per DGE queue
        # so both land ~in parallel with the wsl loads
        xtw_sb = {}
        for m, xtw in enumerate([xtw_t, xtw_f]):
            t = big.tile([128, 2 * (N + ET)], bf16, tag=f"xtw{m}")
            nc.scalar.dma_start(t[:], xtw[:])
            xtw_sb[m] = t

        # one [128, NJT*BLK] tile + ONE DMA per branch: 32 separate DMAs
        # cost ~600ns of DGE descriptor generation EACH on the sync queue
        # (~20us of serialization) and give the pte chain 16 staggered
        # semaphore deps
        wsl_sb = {}
        for m, wsl in enumerate([wsl_t, wsl_f]):
            t = big.tile([128, NJT * BLK], bf16, tag=f"wsl{m}")
            nc.sync.dma_start(t[:], wsl[:])
            wsl_sb[m] = [t[:, kt * BLK:(kt + 1) * BLK] for kt in range(NJT)]

        i128b_sb = cb[:, 0:128]
        onesb_sb = cb[:, 128:129]
        sel64_sb = cb[0:FD, 129:257]
        ebig_sb = cb[:, 257:511]
        crf = cst.tile([1, 128], f32)
        nc.sync.dma_start(crf[:], crowf[:])
        onesrow_sb = crf[:, 0:128]

        # ---- remaining packed constants (ACT queue, behind xtw) ----
        cf = cst.tile([128, 132], f32)
        nc.scalar.dma_start(cf[:], cf128[:])
        i128f_sb = cf[:, 0:128]
        bias_c = cf[:, 128:129]
        bias_o = cf[0:C, 129:130]
        bias_tf = [cf[0:ET, 130:131], cf[0:ET, 131:132]]
        cr = cst.tile([1, 128 + N], bf16)
        nc.scalar.dma_start(cr[:], crow[:])
        ones1b_sb = cr[:, 0:128]
        epsrow_sb = cr[:, 128:128 + N]
        ebig8_sb = cst.tile([128, 384], fp8)
        nc.scalar.dma_start(ebig8_sb[:], ebig8[:])

        # PE pstate warm-up: dummy matmuls on a memset tile (no DMA wait,
        # so the warm-up finishes before the first real operands land)
        wsrc = wk.tile([128, 128], bf16, tag="wsrc")
        nc.vector.memset(wsrc[:], 1.0)
        for wi in range(10):
            pwu = ps.tile([128, 128], f32, tag="pgen")
            nc.tensor.matmul(pwu[:], wsrc[:], wsrc[:],
                             start=True, stop=True)

        # warm the Lrelu ACT table while DMAs land (hides the ~1.3us
        # table load that would otherwise precede the te activation)
        lwarm = wk.tile([1, 1], f32, tag="lwarm")
        nc.scalar.activation(lwarm[:], crf[:, 0:1], Act.Lrelu, alpha=0.01)

        pz_all = {}
        for m in range(2):
            xts = [xtw_sb[m][:, 0:N], xtw_sb[m][:, N + ET:2 * N + ET]]
            wms = [xtw_sb[m][:, N:N + ET],
                   xtw_sb[m][:, 2 * N + ET:2 * (N + ET)]]
            pz = psz.tile([128, NJT * ET], f32, tag=f"pz{m}")
            for jt in range(NJT):
                sl = slice(jt * ET, (jt + 1) * ET)
                for k in range(2):
                    nc.tensor.matmul(pz[:, sl],
                                     xts[k][:, jt * 128:(jt + 1) * 128],
                                     wms[k], start=(k == 0), stop=(k == 1))
            pz_all[m] = pz

        # te^T / fe^T [32, 256] bf16 -> g2_in
        # (adjacency normalization is folded into wsl on the host, so
        #  te = lrelu(Z^T @ wsl + b) directly); the two branches'
        # accumulation chains interleave across two PSUM banks so the
        # PE pipelines them instead of stalling on each accumulate
        zbs = {}
        for m in range(2):
            zbc = []
            for q in range(4):
                zq = big.tile([128, 4 * ET], bf16, tag=f"zb{m}_{q}")
                nc.vector.tensor_copy(zq[:],
                                      pz_all[m][:, q * 4 * ET:
                                                 (q + 1) * 4 * ET])
                zbc.append(zq)
            zbs[m] = zbc
        pte0 = ps.tile([ET, BLK], f32, tag="pgen")
        pte1 = ps.tile([ET, BLK], f32, tag="pgen")
        ptes = [pte0, pte1]
        for kt in range(NJT):
            for m in range(2):
                nc.tensor.matmul(ptes[m][:],
                                 zbs[m][kt // 4][:, (kt % 4) * ET:
                                                 (kt % 4 + 1) * ET],
                                 wsl_sb[m][kt],
                                 start=(kt == 0), stop=(kt == NJT - 1))
        te_sb = []
        for m in range(2):
            te = wk.tile([ET, BLK], bf16, tag=f"teT{m}")
            nc.scalar.activation(te[:], ptes[m][:], Act.Lrelu,
                                 bias=bias_tf[m], alpha=0.01)
            te_sb.append(te)
            nc.sync.dma_start(g2_in[m * ET:(m + 1) * ET, :], te[:])

        # slab from OWN te/fe (read back from g2_in DRAM, straight
        # slices). slabW[p = r*64 + m*32 + e, q] = g2_in[m*32 + e, q]
        slabW = cst.tile([128, BLK], bf16)
        for r in range(2):
            nc.sync.dma_start(slabW[r * FD:(r + 1) * FD, :], g2_in[:])
        # slab32b[p, c] = bf16(S * fd_own[k, 2c+r])  — bf16-rounded so the
        # diagonal |in0 - slab| is exactly 0 (in0 is bf16(S*fd) too).
        slab32 = cst.tile([128, 128], bf16)
        for r in range(2):
            half = slice(r * 64, (r + 1) * 64)
            src = slabW[half, :].rearrange("k (c r2) -> r2 k c", r2=2)[r]
            nc.vector.tensor_scalar(slab32[half, :], src, SSC, None, Alu.mult)
        slabf = cst.tile([128, 128], f32)
        nc.vector.tensor_copy(slabf[:], slab32[:])
        negslab = cst.tile([128, 128], f32)
        nc.vector.tensor_scalar(negslab[:], slab32[:], -1.0, None, Alu.mult)

        # eps seeds for f=0 (constants only — run during the g2 window)
        psd_f0 = []
        for ch in range(NCHUNK):
            pcht = psd.tile([128, 512], f32, tag=f"pd{ch}")
            psd_f0.append(pcht)
            sl = slice(ch * 512, (ch + 1) * 512)
            nc.tensor.matmul(pcht[:], ones1b_sb, epsrow_sb[0:1, sl],
                             start=True, stop=False)

        nc.gpsimd.collective_compute(
            "AllGather", Alu.bypass, replica_groups=RG,
            ins=[g2_in.ap().opt()], outs=[g2_out.ap().opt()])

        # fdT in 4 separate chunk tiles so in0 chunk ch only waits its DMA
        fdT_view = g2_out.ap().rearrange("(c k) q -> k c q", c=NCORE, k=FD)
        fdT_ch = []
        for ch in range(NCHUNK):
            t = big.tile([128, 512], bf16, tag=f"fdT{ch}")
            nc.sync.dma_start(t[:FD, :], fdT_view[:, 2 * ch:2 * ch + 2, :])
            nc.sync.dma_start(t[FD:FD + C, :], ohT[:, ch * 512:(ch + 1) * 512])
            fdT_ch.append(t)
        m1t = cst.tile([128, 2 * N], bf16)
        nc.scalar.dma_start(m1t[:], m1[:])
        m1_sb = [m1t[:, 0:N], m1t[:, N:2 * N]]

        # in0[p = r*64+k, j] = bf16(S * fd[j, k]) via PE broadcast;
        # scale by S during the PSUM->SBUF copy
        in0 = big.tile([128, N], bf16, tag="in0")
        for ch in range(NCHUNK):
            sl = slice(ch * 512, (ch + 1) * 512)
            pin = ps.tile([128, 512], f32, tag="pgen")
            nc.tensor.matmul(pin[:], sel64_sb, fdT_ch[ch][:FD, :],
                             start=True, stop=True)
            # alternate the scale-copy across ACT/DVE so the four chunk
            # copies run pairwise in parallel (distance start gates on in0)
            if ch % 2 == 0:
                nc.scalar.activation(in0[:, sl], pin[:], Act.Copy, bias=0.0,
                                     scale=SSC)
            else:
                nc.vector.tensor_scalar(in0[:, sl], pin[:], SSC, None,
                                        Alu.mult)

        # ---- distance row-blocks ----
        i16 = mybir.dt.int16
        zer = cst.tile([128, N], bf16)
        if FUSE == "stt" or D8_N > 0:
            nc.vector.memset(zer[:], 0.0)

        def emit_dve_abs(dst, cg):
            if FUSE == "stt":
                # |in0 - s| = abs_max(in0 - s, 0) via scalar-tensor-tensor
                nc.vector.scalar_tensor_tensor(
                    dst, in0[:], slabf[:, cg:cg + 1], zer[:],
                    Alu.subtract, Alu.abs_max)
            else:
                nc.vector.tensor_scalar(dst, in0[:], slabf[:, cg:cg + 1],
                                        None, Alu.subtract)
                nc.vector.tensor_scalar(dst.bitcast(i16), dst.bitcast(i16),
                                        0x7FFF, None, Alu.bitwise_and)

        pattern = _fill_pattern()
        dwrow = big.tile([1, BLK], f32, tag="dwrow")
        # aw_rows in bf16: the diagonal is M1-corrected to ~1/S BEFORE the
        # rowsum (no large-number DIAG_CORR cancellation), so deg survives
        # bf16 and the 16-bit XBAR DMA transpose becomes legal
        aw_rows = []
        for f in range(2):
            awr_t = big.tile([128, N], bf16, tag=f"awr{f}")
            aw_rows.append(awr_t)
        aw_cols = []
        if PODR:
            for ktp in range(NJT // 2):
                awc_t = big.tile([128, 2 * BLK], fp8, tag=f"awc{ktp}")
                aw_cols.append(awc_t)
        else:
            for kt in range(NJT):
                awc_t = big.tile([128, BLK], bf16, tag=f"awc{kt}")
                aw_cols.append(awc_t)
        dwcol = []

        def emit_deg(f, dwr_=None):
            # deg rowsum; the diag is already M1-corrected to ~1/S, so the
            # sum is exactly (deg+1)/S — no correction constant needed.
            # For f=1 the caller passes per-chunk-accumulated partials.
            # f=0 (mid-distance) rides on ACT accum_out — DVE is the
            # binding engine there (97.9% busy) while ACT has slack; the
            # copy target is the dead f0 aw32 tile (no fill-pool theft).
            if dwr_ is None:
                dwr_ = big.tile([128, 1], f32, tag=f"dwcr{f}")
                nc.scalar.activation(aw32_f[f][:], aw_rows[f][:], Act.Copy,
                                     accum_out=dwr_[:])
            pr = ps.tile([1, 128], f32, tag="pgen")
            nc.tensor.matmul(pr[:], dwr_[:], i128f_sb, start=True, stop=True)
            nc.vector.tensor_copy(dwrow[:, f * 128:(f + 1) * 128], pr[:])
            nc.sync.dma_start(g3h_in[f][:], dwrow[:, f * 128:(f + 1) * 128])
            nc.gpsimd.collective_compute(
                "AllGather", Alu.bypass, replica_groups=RG,
                ins=[g3h_in[f].ap().opt()], outs=[g3h_out[f].ap().opt()])

        aw32_f = {}

        def emit_m1(f):
            # M1 diag fix; also performs the f32 -> bf16 conversion of
            # the recip output (recip itself must write fp32).
            # f=0 runs mid-distance where DVE is the binding engine
            # (~98% busy) — run it on the otherwise idle GPSIMD there.
            eng = nc.gpsimd if f == 0 else nc.vector
            eng.tensor_tensor(aw_rows[f][:], aw32_f[f][:],
                              m1_sb[f], Alu.mult)

        def emit_unit(unit, c, f, pch, is_last):
            cg = f * 64 + c
            if unit[0] == 's':
                dve = unit[1] == 'db'
                pool = tpd if dve else tpg
                t = pool.tile([128, N], bf16, tag="tb")
                if dve:
                    emit_dve_abs(t[:], cg)
                else:
                    nc.scalar.activation(t[:], in0[:], Act.Abs,
                                         bias=negslab[:, cg:cg + 1])
                s = 126 - 2 * c
                for ch in range(NCHUNK):
                    sl = slice(ch * 512, (ch + 1) * 512)
                    nc.tensor.matmul(pch[ch][:], ebig_sb[:, s:s + 128],
                                     t[:, sl], start=False, stop=is_last)
                return c + 1
            # fp8 pair at columns c, c+1
            t8 = tp8.tile([128, 2 * N], fp8, tag="t8")
            for h, eng in enumerate(unit[1:3]):
                half = t8[:, h * N:(h + 1) * N]
                cgh = cg + h
                if eng == 'd8':
                    nc.vector.scalar_tensor_tensor(
                        half, in0[:], slabf[:, cgh:cgh + 1], zer[:],
                        Alu.subtract, Alu.abs_max)
                elif eng == 'g8':
                    nc.gpsimd.tensor_scalar(half, in0[:],
                                            slabf[:, cgh:cgh + 1], 0.0,
                                            Alu.subtract, Alu.abs_max)
                else:
                    nc.scalar.activation(half, in0[:], Act.Abs,
                                         bias=negslab[:, cgh:cgh + 1])
            s8 = 126 - 2 * c
            w8 = ebig8_sb[:, s8:s8 + 256].rearrange("p (two m) -> p two m",
                                                    two=2)
            t8v = t8[:].rearrange("p (two n) -> p two n", two=2)
            for ch in range(NCHUNK):
                nc.tensor.matmul(pch[ch][:], w8,
                                 t8v[:, :, ch * 512:(ch + 1) * 512],
                                 start=False, stop=is_last,
                                 perf_mode=DR)
            return c + 2

        for f in range(2):
            if f == 0:
                pch = psd_f0
            else:
                pch = []
                for ch in range(NCHUNK):
                    # chunks 0-1 reuse the psz banks (dead since the
                    # head), so their eps seeds need not wait f0 recips
                    if ch < 2:
                        pcht = psz.tile([128, 512], f32, tag=f"pz{ch}")
                    else:
                        pcht = psd.tile([128, 512], f32, tag=f"pd{ch - 2}")
                    pch.append(pcht)
                # eps seed: psum = S*EPS everywhere (start=True)
                for ch in range(NCHUNK):
                    sl = slice(ch * 512, (ch + 1) * 512)
                    nc.tensor.matmul(pch[ch][:], ones1b_sb,
                                     epsrow_sb[0:1, sl],
                                     start=True, stop=False)
            c = 0
            for ui, unit in enumerate(pattern):
                c = emit_unit(unit, c, f, pch, ui == len(pattern) - 1)
                if f == 1 and ui == len(pattern) // 3:
                    emit_m1(0)
                    emit_deg(0)
            assert c == 64
            # 1/(S*(dist+eps)) = Ahw/S rows (per-chunk: next fill's eps
            # seed of chunk ch only waits recip of chunk ch)
            aw32 = big.tile([128, N], f32, tag="aw32")
            aw32_f[f] = aw32
            if f == 0:
                for ch in range(NCHUNK):
                    sl = slice(ch * 512, (ch + 1) * 512)
                    nc.vector.reciprocal_approx_fast(out=aw32[:, sl],
                                                     in_=pch[ch][:])
            else:
                # per-chunk recip -> M1 -> ACT rowsum pipeline so the g3b
                # trigger fires right after the last chunk instead of
                # after two serial full-width ops
                dwp = big.tile([128, NCHUNK], f32, tag="dwp")
                for ch in range(NCHUNK):
                    sl = slice(ch * 512, (ch + 1) * 512)
                    nc.vector.reciprocal_approx_fast(out=aw32[:, sl],
                                                     in_=pch[ch][:])
                    nc.vector.tensor_tensor(aw_rows[1][:, sl],
                                            aw32[:, sl],
                                            m1_sb[1][:, sl], Alu.mult)
                    scr = tpd.tile([128, 512], bf16, tag="tb")
                    nc.scalar.activation(scr[:], aw_rows[1][:, sl],
                                         Act.Copy,
                                         accum_out=dwp[:, ch:ch + 1])
                dwr1 = big.tile([128, 1], f32, tag="dwcr1")
                nc.vector.tensor_reduce(dwr1[:], dwp[:],
                                        mybir.AxisListType.X, Alu.add)
                emit_deg(1, dwr1)

        # dwr128 broadcast from the local deg row (overlaps g3)
        rcr = wk.tile([1, BLK], f32, tag="rcr")
        nc.vector.reciprocal_approx_fast(out=rcr[:], in_=dwrow[:])
        dwr = wk.tile([1, BLK], f32, tag="dwr")
        nc.scalar.activation(dwr[:], rcr[:], Act.Sqrt)
        pb128 = ps.tile([128, BLK], f32, tag="pgen")
        nc.tensor.matmul(pb128[:], onesrow_sb, dwr[:],
                         start=True, stop=True)
        dwr128 = big.tile([128, BLK], f32, tag="dwr128")
        nc.vector.tensor_copy(dwr128[:], pb128[:])

        # transposes -> aw_cols plus F' = feats @ Wc (PE idles during
        # g3b). Even j-tiles first: they feed the h=0 half of po, which
        # only waits on the early hidden g3a collective. The psum->sbuf
        # copies alternate DVE/ACT so the copy tail runs on two engines.
        wc_sb = cst.tile([FD + C, EC], bf16)
        nc.sync.dma_start(wc_sb[:], wc[:])
        fp_all = big.tile([128, NJT * EC], bf16, tag="fpall")

        def emit_cols(kt, alt):
            # PE transposes (XBAR DMA transpose costs ~1.2us of
            # sequencer-side descriptor generation each — measured net
            # loss); psum->sbuf copies alternate DVE/ACT
            for f in range(2):
                # rotate transpose psum across pgen AND the dead psz
                # banks: depth-4 pipeline instead of depth-2
                if f == 0:
                    pt = ps.tile([128, 128], bf16, tag="pgen")
                else:
                    pt = psz.tile([128, 128], bf16, tag=f"pz{kt % 2}")
                nc.tensor.transpose(pt[:],
                                    aw_rows[f][:, kt * 128:(kt + 1) * 128],
                                    i128b_sb)
                dst = aw_cols[kt][:, f * 128:(f + 1) * 128]
                if (alt + f) % 2 == 0:
                    nc.vector.tensor_copy(dst, pt[:])
                else:
                    nc.scalar.copy(dst, pt[:])
            p = ps.tile([128, EC], f32, tag="pgen")
            nc.tensor.matmul(
                p[:],
                fdT_ch[kt // 4][:FD + C, (kt % 4) * 128:(kt % 4 + 1) * 128],
                wc_sb[:], start=True, stop=True)
            if alt % 2 == 0:
                nc.vector.tensor_copy(fp_all[:, kt * EC:(kt + 1) * EC], p[:])
            else:
                nc.scalar.copy(fp_all[:, kt * EC:(kt + 1) * EC], p[:])

        # deg/disw per f-half: the h=0 half only depends on the early
        # (mid-distance) g3a collective, so its 8 po matmuls run before
        # g3b even lands; only the h=1 half waits for g3b. Emission
        # order (even cols -> h0 chain -> odd cols -> h1 chain) keeps
        # the h0 chain ahead of the odd copies in every engine queue.
        # po lives in the psd pool (pd0's bank is free after the last
        # recip) so the pgen rotation can't recycle it mid-accumulation.
        po = psd.tile([EC, BLK], f32, tag="pd0")
        fp_v = fp_all[:].rearrange("p (c two e) -> two p c e", two=2,
                                   c=NCORE, e=EC)

        def emit_half(h):
            dgh = big.tile([128, NCORE], f32, tag=f"dgw{h}")
            nc.sync.dma_start(dgh[:],
                              g3h_out[h].ap().rearrange("c p -> p c"))
            rch = big.tile([128, NCORE], f32, tag=f"rcw{h}")
            nc.vector.reciprocal_approx_fast(out=rch[:], in_=dgh[:])
            dish = big.tile([128, NCORE], bf16, tag=f"disw{h}")
            nc.scalar.activation(dish[:], rch[:], Act.Sqrt)
            dwv = dish[:]
            rw = big.tile([128, NCORE * EC], bf16, tag=f"rw{h}")
            dwrep = bass.AP(tensor=dwv.tensor, offset=dwv.offset,
                            ap=[[NCORE, 128], [1, NCORE], [0, EC]])
            nc.vector.tensor_tensor(
                rw[:].rearrange("p (a c) -> p a c", a=NCORE),
                fp_v[h], dwrep, Alu.mult)
            for cc in range(NCORE):
                kt = 2 * cc + h
                nc.tensor.matmul(po[:], rw[:, cc * EC:(cc + 1) * EC],
                                 aw_cols[kt][:],
                                 start=(h == 0 and cc == 0),
                                 stop=(h == 1 and cc == NCORE - 1))

        for i, kt in enumerate(range(0, NJT, 2)):
            emit_cols(kt, i)
        emit_half(0)
        for i, kt in enumerate(range(1, NJT, 2)):
            emit_cols(kt, i)
        emit_half(1)
        # re-warm the Lrelu table under the po matmuls (the Sqrt set
        # load above evicted it; this keeps embT's Lrelu load off the
        # critical path)
        lw2 = wk.tile([1, 1], f32, tag="lwarm2")
        nc.scalar.activation(lw2[:], crf[:, 0:1], Act.Lrelu, alpha=0.01)
        tmp3 = wk.tile([EC, BLK], f32, tag="tmp3")
        nc.vector.tensor_tensor(tmp3[:], po[:], dwr128[:], Alu.mult)
        embT = wk.tile([EC, BLK], bf16, tag="embT")
        nc.scalar.activation(embT[:], tmp3[:], Act.Lrelu, bias=bias_c,
                             alpha=0.01)

        wo_sb = cst.tile([EC, C], bf16)
        nc.sync.dma_start(wo_sb[:], wo[:])
        ph = ps.tile([C, BLK], f32, tag="pgen")
        nc.tensor.matmul(ph[:], wo_sb[:], embT[:], start=True, stop=True)
        yout = wk.tile([C, BLK], f32, tag="yout")
        nc.vector.tensor_scalar(yout[:], ph[:], bias_o, None, Alu.add)
        nc.sync.dma_start(y[:], yout[:])

    nc.finalize()
    return nc


def _host_prep(inputs):
    import ml_dtypes
    bf = ml_dtypes.bfloat16
    f8 = ml_dtypes.float8_e4m3

    ei = np.asarray(inputs["edge_index"])
    wt = np.asarray(inputs["time_edge_weight"], np.float32)
    wf = np.asarray(inputs["freq_edge_weight"], np.float32)
    xt = np.asarray(inputs["time_features"], np.float32)
    xf = np.asarray(inputs["freq_features"], np.float32)
    labels = np.asarray(inputs["labels"])
    num_classes = int(inputs["num_classes"])
    query_size = int(inputs["query_size"])
    n = xt.shape[0]
    assert n == N and num_classes == C

    offdiag = ~np.eye(n, dtype=bool)
    r_can = np.repeat(np.arange(n, dtype=ei.dtype), n - 1)
    cgrid = np.broadcast_to(np.arange(n, dtype=ei.dtype), (n, n))
    c_can = cgrid[offdiag]
    canonical = np.array_equal(ei[0], r_can) and np.array_equal(ei[1], c_can)

    def build_wmat(w):
        # Wmat[src, dst] = w  (Wmat = A^T), plus identity
        Wm = np.zeros((n, n), np.float32)
        if canonical:
            Wm[offdiag] = w
        else:
            A = np.zeros((n, n), np.float32)
            np.add.at(A, (ei[1], ei[0]), np.asarray(w, np.float64))
            np.fill_diagonal(A, 0.0)
            Wm = np.ascontiguousarray(A.T.astype(np.float32))
        Wm[np.arange(n), np.arange(n)] = 1.0  # + I
        # fold the GCN symmetric normalization D^-1/2 (A+I) D^-1/2 in here
        # (depends only on the input edge weights; Wm is symmetric-scaled
        #  so the transpose orientation is unaffected)
        deg = Wm.sum(axis=0, dtype=np.float64)
        dis = 1.0 / np.sqrt(deg)
        return (dis[:, None] * Wm * dis[None, :]).astype(np.float32)

    Wm_t = build_wmat(wt)
    Wm_f = build_wmat(wf)

    cols = np.zeros((num_classes,), np.float32)
    cols[labels] = 1.0
    rowmask = (np.arange(n) < (n - query_size)).astype(np.float32)
    onehotT = np.ascontiguousarray(rowmask[None, :] * cols[:, None])

    # selector: in0[p = r*64+k] = fdT[k]
    s64 = np.zeros((FD, 128), np.float32)
    for r in range(2):
        s64[np.arange(FD), r * FD + np.arange(FD)] = 1.0

    up = (np.arange(128) < 64).astype(np.float32)
    dn = 1.0 - up
    eb = np.zeros((128, 254), np.float32)
    eb[:, 126] = up
    eb[:, 127] = dn
    eb8 = np.zeros((128, 384), np.float32)
    eb8[:, 126] = up
    eb8[:, 127] = dn
    eb8[:, 256] = up
    eb8[:, 257] = dn

    # diag value after recip: 1/bf16(S*EPS); M1 maps it to 1/S
    seps = np.float32(bf(SSC * EPS))
    diagfix = np.float32((1.0 / SSC) / (1.0 / seps))

    def pack_bf(parts, width):
        blob = np.zeros((128, width), np.float32)
        col = 0
        for p in parts:
            r, w = p.shape
            blob[:r, col:col + w] = p
            col += w
        assert col == width
        return blob

    # cb128: i128b | onesb | sel64 | ebig
    cbblob = pack_bf([np.eye(128, dtype=np.float32),
                      np.ones((128, 1), np.float32), s64, eb], 511)
    # cf128: i128f | bc | bo | b_t | b_f
    cfblob = pack_bf([np.eye(128, dtype=np.float32),
                      np.asarray(inputs["bc"], np.float32).reshape(EC, 1),
                      np.asarray(inputs["bo"], np.float32).reshape(C, 1),
                      np.asarray(inputs["bt"], np.float32).reshape(ET, 1),
                      np.asarray(inputs["bf"], np.float32).reshape(ET, 1)],
                     132)
    crblob = np.concatenate([np.ones((1, 128), np.float32),
                             np.full((1, N), SSC * EPS, np.float32)], axis=1)

    def xtw(x, w):
        arr = np.concatenate(
            [np.ascontiguousarray(x.T),
             np.asarray(w, np.float32)], axis=1)
        return np.ascontiguousarray(
            arr.reshape(2, 128, N + ET).transpose(1, 0, 2)
            .reshape(128, 2 * (N + ET))).astype(bf)

    shared = {
        "xtw_t": xtw(xt, inputs["Wt"]),
        "xtw_f": xtw(xf, inputs["Wf"]),
        "wc": np.asarray(inputs["Wc"], np.float32).astype(bf),
        "wo": np.asarray(inputs["Wo"], np.float32).astype(bf),
        "ohT": onehotT.astype(bf),
        "cb128": cbblob.astype(bf),
        "cf128": cfblob,
        "crow": crblob.astype(bf),
        "crowf": np.ones((1, 128), np.float32),
        "ebig8": eb8.astype(f8),
    }

    in_maps = []
    for b in range(NCORE):
        m = dict(shared)
        mm = np.ones((128, 2 * N), np.float32)
        rows = np.arange(128)
        for f in range(2):
            mm[rows, f * N + b * BLK + f * 128 + rows] = diagfix
        m["m1"] = mm.astype(bf)
        def warr(Wm):
            blk = Wm[:, b * BLK:(b + 1) * BLK].reshape(NJT, 128, BLK)
            return np.ascontiguousarray(
                blk.transpose(1, 0, 2).reshape(128, NJT * BLK)).astype(bf)
        m["wsl_t"] = warr(Wm_t)
        m["wsl_f"] = warr(Wm_f)
        in_maps.append(m)
    return in_maps


def _get_program():
    if "nc" not in _CACHE:
        _CACHE["nc"] = _build_program()
    return _CACHE["nc"]


def run(inputs, trace=False):
    from concourse.bass_utils import run_bass_kernel_spmd
    in_maps = _host_prep(inputs)
    nc = _get_program()
    res = run_bass_kernel_spmd(nc, in_maps, core_ids=list(range(NCORE)),
                               trace=trace)
    blocks = [res.results[b]["y"] for b in range(NCORE)]
    out = np.concatenate([blk.T for blk in blocks], axis=0).astype(np.float32)
    return out, res


def kernel(**inputs):
    out, _ = run(inputs, trace=False)
    return out

